# revision 33
# baseline (speedup 1.0000x reference)
"""PixPro loss kernel for 8 Trainium2 NeuronCores.

Data-parallel over batch: 1024 samples -> 128 per core (= SBUF partitions).
Features are host-converted to fp16 (halves HBM traffic; final rel err vs
the f32 reference is ~1e-4, far inside the 2e-2 gate).

Heavy part (cos similarity over 512 channels x 49 grid points): every
accumulating DVE op is a dual-op "Reduce" instruction and runs at 1x
(measured), so the optimal structure is one fused scalar_tensor_tensor
(multiply + channel-accumulate, ~690ns/point) per dot on DVE, with the 98
squares split DVE/ACT (~683 vs ~900ns incl. the ACTIVATION_READ_ACCUMULATOR
slice) to balance the two engines at ~68us each.

Mask part exploits the rank-1 grid structure: gx depends only on the grid
row index, gy only on the column index, so D2[i,j] = dx2[ri,rj] +
dy2[ci,cj] comes from two 7x7 tables via broadcast adds (on the otherwise
idle GPSIMD; the HW ISA caps tensor ops at 3 free dims, hence one op per
ci). Masks are single-op is_lt tensor_scalar (2x_2p mode) emitted
mid-stream; nnz/colsum/rowsum come from tensor_reduce.

Per-core output is [128, 2] = (masked loss contribution, intersection
flag); host does the final psum + divide.
"""

import sys

import numpy as np

if "/opt/trn_rl_repo" not in sys.path:
    sys.path.insert(0, "/opt/trn_rl_repo")

B = 1024
C = 512
S = 7
N = S * S  # 49
NCORES = 8
BP = B // NCORES  # 128 samples per core
NBLK = 7  # 7 blocks of 7 grid points
BLKN = N // NBLK  # 7
THRESH2 = 0.7 * 0.7

# m^2 engine assignment per (block, point-in-block): V=DVE fused STT, A=ACT
# (b^2 always on ACT; DVE takes 30 of the m^2 squares)
M2_TABLE = [
    "VVVVVVV",
    "VVVVVVV",
    "VVVVVVV",
    "VVVVVVV",
    "VVAAAAA",
    "AAAAAAA",
    "AAAAAAA",
]

_t = np.linspace(0.0, 1.0, S).astype(np.float32)
TTAB = np.ascontiguousarray(np.tile(_t[None, :], (BP, 1)))  # [128, 7]

_NC = None


def _emit(tc, d):
    from contextlib import ExitStack

    from concourse import mybir

    nc = tc.nc
    f32 = mybir.dt.float32
    f16 = mybir.dt.float16
    A = mybir.AluOpType
    AX = mybir.AxisListType
    SQ = mybir.ActivationFunctionType.Square

    with ExitStack() as ctx:
        pers = ctx.enter_context(tc.tile_pool(name="pers", bufs=1))

        # ---- tiles ----
        pbm_t = pers.tile([BP, 8], f32, tag="pbm_t")  # xb yb wb hb xm ym wm hm
        fbm_t = pers.tile([BP, 2], f32, tag="fbm_t")
        ttab_t = pers.tile([BP, S], f32, tag="ttab_t")

        bt_sb = pers.tile([BP, N, C], f16, tag="bt_sb")
        mt_sb = pers.tile([BP, N, C], f16, tag="mt_sb")

        scr_v = pers.tile([BP, C], f16, tag="scr_v")  # DVE STT dump
        scr_a = pers.tile([BP, C], f16, tag="scr_a")  # ACT dump

        dot_sb = pers.tile([BP, N], f32, tag="dot_sb")
        nb2 = pers.tile([BP, N], f32, tag="nb2")
        nm2 = pers.tile([BP, N], f32, tag="nm2")

        # mask part
        fh = pers.tile([BP, 2], f32, tag="fh")
        m2f = pers.tile([BP, 2], f32, tag="m2f")
        wh4 = pers.tile([BP, 4], f32, tag="wh4")  # wb h2b wm h2m
        xy4 = pers.tile([BP, 4], f32, tag="xy4")  # xb y2b xm y2m
        tmp4 = pers.tile([BP, 4, S], f32, tag="tmp4")
        g4 = pers.tile([BP, 4, S], f32, tag="g4")  # gxb gyb gxm gym
        dd = pers.tile([BP, 2, S, S], f32, tag="dd")
        dd2 = pers.tile([BP, 2, S, S], f16, tag="dd2")
        D2 = pers.tile([BP, S, S, S, S], f16, tag="D2")  # [ri ci rj cj]
        mk_b = pers.tile([BP, N, N], f16, tag="mk_b")  # [i, j]
        FT = pers.tile([BP, 48, N], f16, tag="FT")  # colsum fold scratch
        mk_m = pers.tile([BP, N, N], f16, tag="mk_m")  # [i, j]
        p2 = pers.tile([BP, 8], f32, tag="p2")
        sum2 = pers.tile([BP, 2], f32, tag="sum2")
        tau2 = pers.tile([BP, 2], f32, tag="tau2")
        cc = pers.tile([BP, 4], f32, tag="cc")
        dq = pers.tile([BP, 4], f32, tag="dq")  # dx dy sw sh
        qq = pers.tile([BP, 4], f32, tag="qq")
        ok = pers.tile([BP, 2], f32, tag="ok")
        inter = pers.tile([BP, 1], f32, tag="inter")
        nnz2 = pers.tile([BP, 2], f32, tag="nnz2")
        colsum_b = pers.tile([BP, N], f32, tag="colsum_b")
        rowsum_m = pers.tile([BP, N], f32, tag="rowsum_m")

        # tail
        den2 = pers.tile([BP, N], f32, tag="den2")
        den = pers.tile([BP, N], f32, tag="den")
        inv = pers.tile([BP, N], f32, tag="inv")
        cos_t = pers.tile([BP, N], f32, tag="cos_t")
        scr49 = pers.tile([BP, N], f32, tag="scr49")
        ss = pers.tile([BP, 2], f32, tag="ss")
        nnzc = pers.tile([BP, 2], f32, tag="nnzc")
        invn = pers.tile([BP, 2], f32, tag="invn")
        l2 = pers.tile([BP, 2], f32, tag="l2")
        lsum = pers.tile([BP, 1], f32, tag="lsum")
        out_sb = pers.tile([BP, 2], f32, tag="out_sb")

        # ---- DMA order tuned for ramp: pbm, first feature chunks, rest ----
        nc.sync.dma_start(pbm_t[:], d["pbm"][:])
        nc.sync.dma_start(bt_sb[:, 0:2, :], d["bt"][:, 0:2, :])
        nc.sync.dma_start(mt_sb[:, 0:2, :], d["mt"][:, 0:2, :])
        nc.sync.dma_start(fbm_t[:], d["fbm"][:])
        nc.sync.dma_start(ttab_t[:], d["ttab"][:])
        for a, b in ((2, 4), (4, 7)):
            nc.sync.dma_start(bt_sb[:, a:b, :], d["bt"][:, a:b, :])
            nc.sync.dma_start(mt_sb[:, a:b, :], d["mt"][:, a:b, :])
        H = 4  # sub-block split: points [0:4) and [4:7)
        for blk in range(1, NBLK):
            n0 = blk * BLKN
            nc.sync.dma_start(bt_sb[:, n0 : n0 + H, :], d["bt"][:, n0 : n0 + H, :])
            nc.sync.dma_start(mt_sb[:, n0 : n0 + H, :], d["mt"][:, n0 : n0 + H, :])
            nc.sync.dma_start(
                bt_sb[:, n0 + H : n0 + BLKN, :], d["bt"][:, n0 + H : n0 + BLKN, :]
            )
            nc.sync.dma_start(
                mt_sb[:, n0 + H : n0 + BLKN, :], d["mt"][:, n0 + H : n0 + BLKN, :]
            )

        # ---- ACT table warmups (Square now, Sqrt preloaded for the tail) ----
        warm = pers.tile([BP, 2], f32, tag="warm")
        nc.scalar.activation(warm[:, 0:1], pbm_t[:, 0:1], SQ)
        nc.scalar.activation(
            warm[:, 1:2], pbm_t[:, 0:1], mybir.ActivationFunctionType.Sqrt
        )

        # ---- mask part (DVE small ops; marginals on GPSIMD) ----
        wcols = pbm_t[:, 2:8:4]  # wb wm
        hcols = pbm_t[:, 3:8:4]  # hb hm
        xcols = pbm_t[:, 0:8:4]  # xb xm
        ycols = pbm_t[:, 1:8:4]  # yb ym

        # flips: y2 = y + h*f ; h2 = h*(1-2f)
        nc.vector.tensor_tensor(fh[:], fbm_t[:], hcols, A.mult)
        nc.vector.tensor_tensor(xy4[:, 1:4:2], ycols, fh[:], A.add)  # y2b y2m
        nc.vector.tensor_scalar(m2f[:], fbm_t[:], -2.0, 1.0, A.mult, A.add)
        nc.vector.tensor_tensor(wh4[:, 1:4:2], hcols, m2f[:], A.mult)  # h2b h2m
        nc.vector.tensor_copy(wh4[:, 0:4:2], wcols)  # wb wm
        nc.vector.tensor_copy(xy4[:, 0:4:2], xcols)  # xb xm

        # grids g4[s, k, t] = xy4[k] + wh4[k]*ttab[t]
        nc.vector.tensor_tensor(
            tmp4[:],
            wh4[:].unsqueeze(2).broadcast_to([BP, 4, S]),
            ttab_t[:].unsqueeze(1).broadcast_to([BP, 4, S]),
            A.mult,
        )
        nc.vector.tensor_tensor(
            g4[:], tmp4[:], xy4[:].unsqueeze(2).broadcast_to([BP, 4, S]), A.add
        )

        # dd[s, 0, a, b] = gxb[a]-gxm[b]; dd[s, 1, a, b] = gyb[a]-gym[b]
        nc.vector.tensor_tensor(
            dd[:],
            g4[:, 0:2, :].unsqueeze(3).broadcast_to([BP, 2, S, S]),
            g4[:, 2:4, :].unsqueeze(2).broadcast_to([BP, 2, S, S]),
            A.subtract,
        )
        nc.vector.tensor_tensor(dd2[:], dd[:], dd[:], A.mult)

        # D2[ri, ci, rj, cj] = dx2[ri, rj] + dy2[ci, cj]  (on GPSIMD;
        # one op per ci: the TPB ISA caps tensor ops at 3 free dims)
        dx2b = dd2[:, 0, :, :].unsqueeze(3).broadcast_to([BP, S, S, S])
        for ci in range(S):
            dy2c = (
                dd2[:, 1, ci, :]
                .unsqueeze(1)
                .unsqueeze(1)
                .broadcast_to([BP, S, S, S])
            )
            nc.gpsimd.tensor_tensor(D2[:, :, ci, :, :], dx2b, dy2c, A.add)

        # tau^2 = 0.49*(w^2+h^2) per side
        nc.vector.tensor_tensor(p2[:], pbm_t[:], pbm_t[:], A.mult)
        nc.vector.tensor_tensor(sum2[:], p2[:, 2:8:4], p2[:, 3:8:4], A.add)
        nc.vector.tensor_scalar_mul(tau2[:], sum2[:], THRESH2)

        # masks run on DVE mid-heavy-stream (emitted at blk==2 below)

        # intersection test (squared form, no Abs):
        # (2|c1-c2|)^2 < (w1+w2)^2 per axis
        nc.vector.scalar_tensor_tensor(
            cc[:, 0:2], pbm_t[:, 2:4], 0.5, pbm_t[:, 0:2], A.mult, A.add
        )
        nc.vector.scalar_tensor_tensor(
            cc[:, 2:4], pbm_t[:, 6:8], 0.5, pbm_t[:, 4:6], A.mult, A.add
        )
        nc.vector.tensor_tensor(dq[:, 0:2], cc[:, 0:2], cc[:, 2:4], A.subtract)
        nc.vector.tensor_tensor(dq[:, 2:4], pbm_t[:, 2:4], pbm_t[:, 6:8], A.add)
        nc.vector.tensor_tensor(qq[:], dq[:], dq[:], A.mult)
        nc.vector.scalar_tensor_tensor(
            ok[:], qq[:, 0:2], 4.0, qq[:, 2:4], A.mult, A.is_lt
        )
        nc.vector.tensor_tensor(inter[:], ok[:, 0:1], ok[:, 1:2], A.mult)

        # ---- heavy loop ----
        for blk in range(NBLK):
            n0 = blk * BLKN
            for j in range(BLKN):
                n = n0 + j
                bn = bt_sb[:, n, :]
                mn = mt_sb[:, n, :]
                # dot: fused STT (1x) on DVE
                nc.vector.scalar_tensor_tensor(
                    scr_v[:], bn, 1.0, mn, A.mult, A.mult,
                    accum_out=dot_sb[:, n : n + 1],
                )
                # |b|^2 on ACT
                nc.scalar.activation(
                    scr_a[:], bn, SQ, accum_out=nb2[:, n : n + 1]
                )
                # |m|^2 per table
                if M2_TABLE[blk][j] == "A":
                    nc.scalar.activation(
                        scr_a[:], mn, SQ, accum_out=nm2[:, n : n + 1]
                    )
                else:  # V: fused STT square on DVE
                    nc.vector.scalar_tensor_tensor(
                        scr_v[:], mn, 1.0, mn, A.mult, A.mult,
                        accum_out=nm2[:, n : n + 1],
                    )
            if blk == 3:
                # masks (+ marginals) mid-stream on DVE, well after the GPSIMD
                # D2 tiles are ready (GPS inter-op stalls delay D2 to ~23us)
                nc.vector.tensor_scalar(
                    mk_b[:], D2[:], tau2[:, 0:1], None, A.is_lt
                )
                nc.vector.tensor_scalar(
                    mk_m[:], D2[:], tau2[:, 1:2], None, A.is_lt
                )
                # colsum_b[j] = sum_i mask_b[i, j]: fold the outer i axis with
                # 2x TT adds (packed inner j) instead of a strided 1x reduce.
                # Exact in f16 (integer counts <= 49).
                nc.vector.tensor_tensor(
                    FT[:, 0:24, :], mk_b[:, 0:24, :], mk_b[:, 24:48, :], A.add
                )
                nc.vector.tensor_tensor(
                    FT[:, 24:36, :], FT[:, 0:12, :], FT[:, 12:24, :], A.add
                )
                nc.vector.tensor_tensor(
                    FT[:, 36:42, :], FT[:, 24:30, :], FT[:, 30:36, :], A.add
                )
                nc.vector.tensor_tensor(
                    FT[:, 42:45, :], FT[:, 36:39, :], FT[:, 39:42, :], A.add
                )
                nc.vector.tensor_tensor(
                    FT[:, 45:46, :], FT[:, 42:43, :], FT[:, 43:44, :], A.add
                )
                nc.vector.tensor_tensor(
                    FT[:, 46:47, :], FT[:, 45:46, :], FT[:, 44:45, :], A.add
                )
                nc.vector.scalar_tensor_tensor(
                    colsum_b[:], FT[:, 46, :], 1.0, mk_b[:, 48, :],
                    A.mult, A.add,
                )
                nc.vector.tensor_reduce(rowsum_m[:], mk_m[:], AX.X, A.add)
                nc.vector.tensor_reduce(nnz2[:, 0:1], colsum_b[:], AX.X, A.add)
                nc.vector.tensor_reduce(nnz2[:, 1:2], rowsum_m[:], AX.X, A.add)

        # ---- tail ----
        nc.vector.tensor_tensor(den2[:], nb2[:], nm2[:], A.mult)
        nc.scalar.activation(den[:], den2[:], mybir.ActivationFunctionType.Sqrt)
        nc.vector.reciprocal_approx_fast(inv[:], den[:])
        nc.vector.tensor_tensor(cos_t[:], dot_sb[:], inv[:], A.mult)

        nc.vector.scalar_tensor_tensor(
            scr49[:], cos_t[:], 1.0, colsum_b[:], A.mult, A.mult,
            accum_out=ss[:, 0:1],
        )
        nc.vector.scalar_tensor_tensor(
            scr49[:], cos_t[:], 1.0, rowsum_m[:], A.mult, A.mult,
            accum_out=ss[:, 1:2],
        )

        nc.vector.tensor_scalar_max(nnzc[:], nnz2[:], 1.0)
        nc.vector.reciprocal(invn[:], nnzc[:])
        nc.vector.tensor_tensor(l2[:], ss[:], invn[:], A.mult)
        nc.vector.scalar_tensor_tensor(
            lsum[:], l2[:, 0:1], 1.0, l2[:, 1:2], A.mult, A.add
        )
        nc.vector.tensor_tensor(out_sb[:, 0:1], lsum[:], inter[:], A.mult)
        nc.vector.tensor_copy(out_sb[:, 1:2], inter[:])

        nc.sync.dma_start(d["o"][:], out_sb[:])


def build(debug=False):
    import concourse.bacc as bacc
    import concourse.tile as tile
    from concourse import mybir

    nc = bacc.Bacc(
        "TRN2",
        target_bir_lowering=False,
        debug=debug,
        enable_asserts=False,
        num_devices=NCORES,
    )
    f32 = mybir.dt.float32
    f16 = mybir.dt.float16
    d = {
        "bt": nc.dram_tensor("bt", [BP, N, C], f16, kind="ExternalInput").ap(),
        "mt": nc.dram_tensor("mt", [BP, N, C], f16, kind="ExternalInput").ap(),
        "pbm": nc.dram_tensor("pbm", [BP, 8], f32, kind="ExternalInput").ap(),
        "fbm": nc.dram_tensor("fbm", [BP, 2], f32, kind="ExternalInput").ap(),
        "ttab": nc.dram_tensor("ttab", [BP, S], f32, kind="ExternalInput").ap(),
        "o": nc.dram_tensor("o", [BP, 2], f32, kind="ExternalOutput").ap(),
    }
    with tile.TileContext(nc) as tc:
        _emit(tc, d)
    nc.compile()
    return nc


def make_in_maps(base, moment, p_base, p_moment, f_base, f_moment):
    pbm_full = np.concatenate(
        [np.asarray(p_base, dtype=np.float32), np.asarray(p_moment, dtype=np.float32)],
        axis=1,
    )
    fbm_full = np.concatenate(
        [np.asarray(f_base, dtype=np.float32), np.asarray(f_moment, dtype=np.float32)],
        axis=1,
    )
    in_maps = []
    for k in range(NCORES):
        sl = slice(k * BP, (k + 1) * BP)
        bt = np.ascontiguousarray(
            np.asarray(base[sl], dtype=np.float32)
            .reshape(BP, C, N)
            .transpose(0, 2, 1)
            .astype(np.float16)
        )
        mt = np.ascontiguousarray(
            np.asarray(moment[sl], dtype=np.float32)
            .reshape(BP, C, N)
            .transpose(0, 2, 1)
            .astype(np.float16)
        )
        in_maps.append(
            {
                "bt": bt,
                "mt": mt,
                "pbm": np.ascontiguousarray(pbm_full[sl]),
                "fbm": np.ascontiguousarray(fbm_full[sl]),
                "ttab": TTAB,
            }
        )
    return in_maps


def reduce_outputs(per_core_outs):
    """per_core_outs: list of [128, 2] arrays -> final scalar loss."""
    allo = np.concatenate([np.asarray(o, dtype=np.float64) for o in per_core_outs])
    pos = allo[:, 0].sum()
    cnt = allo[:, 1].sum()
    return np.asarray(-pos / max(cnt, 1.0), dtype=np.float32)


def kernel(base, moment, p_base, p_moment, f_base, f_moment, _trace=False):
    global _NC
    from concourse.bass_utils import run_bass_kernel_spmd

    if _NC is None:
        _NC = build()
    in_maps = make_in_maps(base, moment, p_base, p_moment, f_base, f_moment)
    res = run_bass_kernel_spmd(_NC, in_maps, core_ids=list(range(NCORES)), trace=_trace)
    out = reduce_outputs([r["o"] for r in res.results])
    if _trace:
        return out, res
    return out
